# revision 1
# baseline (speedup 1.0000x reference)
"""2-layer GCN (GEMM -> COO SpMM -> ReLU -> GEMM -> SpMM) on 8 trn2 NeuronCores.

Design (row-sharded, transpose-free):
  - Core m owns node rows [m*RPC, (m+1)*RPC); padded to RPAD=NB*128 rows on
    device (pad rows never referenced by gathers; dropped on host).
  - GEMM1: Z1 = X @ W1 + b1 per-core (node-major), bf16, AllGather -> Z1_full.
  - SpMM: per 128-row block, edges sorted by col, split into 4 col-quartile
    segments so gather indices fit int16 relative to a QBASE-row view of
    Z_full.  dma_gather (non-transpose) emits slot-major [128 slots, 128 f]
    chunks == matmul lhsT directly.  S[slot, row] = (iota==row_local)*val is
    one DVE tensor_scalar per chunk.  PE accumulates
    out^T[feats, rows] += G^T @ S in PSUM over a block's Q*CAP_CH chunks.
  - out^T feature-major == lhsT layout for the next GEMM (no transposes).
  - Output written feature-major [128, RPAD] f32; host transposes + trims.

SPMD: one program for 8 cores; fixed slot layout (CAP_CH chunks of 128 per
(block, quartile), padded with idx=0/val=0), per-core data varies only in
input tensors.  DMA-instruction count before fan-in points is minimized:
HW limits sync-waits per instruction and Tile round-robins each DMA onto
one of 8 HWDGE semaphore lanes.
"""

import sys

import numpy as np
import ml_dtypes

_TRN_REPO = "/opt/trn_rl_repo"
if _TRN_REPO not in sys.path:
    sys.path.insert(0, _TRN_REPO)

import concourse.bass as bass
import concourse.tile as tile
from concourse import bacc, mybir
from concourse.bass_utils import run_bass_kernel_spmd

BF16 = mybir.dt.bfloat16
F32 = mybir.dt.float32
I16 = mybir.dt.int16


class Cfg:
    def __init__(self, n_nodes, in_size, hidden, out_size,
                 cap_ch=5, group_blocks=5):
        self.M = 8
        self.NN = n_nodes
        self.IN = in_size
        self.HID = hidden
        self.OUT = out_size
        assert n_nodes % self.M == 0
        self.RPC = n_nodes // self.M          # real rows per core
        self.BL = 128
        self.NB = (self.RPC + 127) // 128
        self.RPAD = self.NB * 128             # padded rows per core
        self.NNP = self.M * self.RPAD         # padded global nodes
        self.Q = 4
        assert self.NNP % self.Q == 0
        self.QBASE = self.NNP // self.Q
        assert self.QBASE <= 32768
        self.CAP_CH = cap_ch
        self.CAP = cap_ch * 128
        self.GB = group_blocks
        self.KIN = in_size // 128
        assert in_size % 128 == 0 and hidden == 128 and out_size == 128


FULL = Cfg(100000, 256, 128, 128)


def build_plan(cfg, row, col, vals):
    row = np.asarray(row).astype(np.int64)
    col = np.asarray(col).astype(np.int64)
    vals = np.asarray(vals).astype(np.float32)
    # remap cols into padded node space
    colp = (col // cfg.RPC) * cfg.RPAD + (col % cfg.RPC)

    # adaptive per-(block, quartile) capacity: scan max segment first
    need = 0
    for m in range(cfg.M):
        sel = (row // cfg.RPC) == m
        er0 = row[sel] - m * cfg.RPC
        key = (er0 // cfg.BL) * cfg.Q + colp[sel] // cfg.QBASE
        if key.size:
            need = max(need, int(np.bincount(key.astype(np.int64)).max()))
    cap_ch = max(cfg.CAP_CH, -(-need // 128))
    if cap_ch != cfg.CAP_CH:
        cfg.CAP_CH = cap_ch
        cfg.CAP = cap_ch * 128

    groups = [list(range(g, min(g + cfg.GB, cfg.NB)))
              for g in range(0, cfg.NB, cfg.GB)]
    slot_off = {}
    insts = []  # (q, slot_offset, n_slots) per (group, quartile)
    off = 0
    for blist in groups:
        for q in range(cfg.Q):
            ioff = off
            for b in blist:
                slot_off[(b, q)] = off
                off += cfg.CAP
            insts.append((q, ioff, off - ioff))
    nslot = off
    nchunk = nslot // 128

    per_core = []
    max_seg = 0
    for m in range(cfg.M):
        sel = (row // cfg.RPC) == m
        er = (row[sel] - m * cfg.RPC).astype(np.int64)
        ec = colp[sel]
        ev = vals[sel]
        blk = er // cfg.BL
        order = np.lexsort((ec, blk))
        er, ec, ev, blk = er[order], ec[order], ev[order], blk[order]

        idx16 = np.zeros(nslot, dtype=np.int16)
        rloc = np.zeros(nslot, dtype=np.float32)
        sval = np.zeros(nslot, dtype=np.float32)

        bstart = np.searchsorted(blk, np.arange(cfg.NB + 1))
        for b in range(cfg.NB):
            i0, i1 = bstart[b], bstart[b + 1]
            ecb = ec[i0:i1]
            qsplit = np.searchsorted(ecb, np.arange(cfg.Q + 1) * cfg.QBASE)
            for q in range(cfg.Q):
                j0, j1 = i0 + qsplit[q], i0 + qsplit[q + 1]
                n = j1 - j0
                max_seg = max(max_seg, n)
                if n > cfg.CAP:
                    raise RuntimeError(
                        f"segment overflow core {m} blk {b} q {q}: "
                        f"{n} > {cfg.CAP}")
                so = slot_off[(b, q)]
                idx16[so:so + n] = (ec[j0:j1] - q * cfg.QBASE).astype(np.int16)
                rloc[so:so + n] = (er[j0:j1] - b * cfg.BL).astype(np.float32)
                sval[so:so + n] = ev[j0:j1]

        idx_w = np.tile(idx16.reshape(-1, 16).T, (8, 1))
        rloc_w = rloc.reshape(nchunk, 128).T.astype(np.float32)
        sval_w = sval.reshape(nchunk, 128).T.astype(np.float32)
        per_core.append(dict(idx=np.ascontiguousarray(idx_w),
                             rloc=np.ascontiguousarray(rloc_w),
                             sval=np.ascontiguousarray(sval_w)))
    return groups, insts, slot_off, nslot, nchunk, per_core, max_seg


def build_program(cfg, groups, insts, slot_off, nslot, nchunk):
    nc = bacc.Bacc("TRN2", target_bir_lowering=False, debug=False,
                   num_devices=cfg.M)

    xt_d = nc.dram_tensor("xt", [cfg.IN, cfg.RPAD], BF16, kind="ExternalInput")
    wcols = cfg.KIN * 128 + 128 + 4 * 128
    wpack_d = nc.dram_tensor("wpack", [128, wcols], BF16, kind="ExternalInput")
    idx_d = nc.dram_tensor("idx", [128, nslot // 16], I16, kind="ExternalInput")
    fcols = 2 * nchunk
    fpack_d = nc.dram_tensor("fpack", [128, fcols], F32, kind="ExternalInput")
    out_d = nc.dram_tensor("out", [128, cfg.RPAD], F32, kind="ExternalOutput")

    z1_loc = nc.dram_tensor("z1_loc", [cfg.RPAD, cfg.HID], BF16)
    z1_full = nc.dram_tensor("z1_full", [cfg.NNP, cfg.HID], BF16)
    z2_loc = nc.dram_tensor("z2_loc", [cfg.RPAD, cfg.OUT], BF16)
    z2_full = nc.dram_tensor("z2_full", [cfg.NNP, cfg.OUT], BF16)

    rg = [list(range(cfg.M))]

    with tile.TileContext(nc) as tc:
        from contextlib import ExitStack
        with ExitStack() as ctx:
            const = ctx.enter_context(tc.tile_pool(name="const", bufs=1))
            xt_pool = ctx.enter_context(tc.tile_pool(name="xt", bufs=8))
            gbuf_pool = ctx.enter_context(tc.tile_pool(name="gbuf", bufs=2))
            s_pool = ctx.enter_context(tc.tile_pool(name="sm", bufs=4))
            ot_pool = ctx.enter_context(tc.tile_pool(name="ot", bufs=8))
            rt_pool = ctx.enter_context(tc.tile_pool(name="rt", bufs=1))
            psum_g = ctx.enter_context(
                tc.tile_pool(name="psum_g", bufs=2, space="PSUM"))
            psum_s = ctx.enter_context(
                tc.tile_pool(name="psum_s", bufs=6, space="PSUM"))

            # ---- resident constants (3 packed loads: bounded sem fan-in) ----
            wpack_sb = const.tile([128, wcols], BF16, tag="wpack",
                                  name="wpacksb")
            nc.sync.dma_start(wpack_sb[:], wpack_d[:, :])
            w1_sb = [wpack_sb[:, k * 128:(k + 1) * 128]
                     for k in range(cfg.KIN)]
            o = cfg.KIN * 128
            w2_sb = wpack_sb[:, o:o + 128]
            b1_sb = wpack_sb[0:1, o + 128:o + 256]
            b2_sb = wpack_sb[0:1, o + 256:o + 384]
            ones_sb = wpack_sb[0:1, o + 384:o + 512]
            iota_sb = wpack_sb[:, o + 512:o + 640]
            idx_sb = const.tile([128, nslot // 16], I16, tag="idx",
                                name="idxsb")
            nc.sync.dma_start(idx_sb[:], idx_d[:, :])
            fpack_sb = const.tile([128, fcols], F32, tag="fpack",
                                  name="fpacksb")
            nc.sync.dma_start(fpack_sb[:], fpack_d[:, :])
            rloc_sb = fpack_sb[:, 0:nchunk]
            sval_sb = fpack_sb[:, nchunk:2 * nchunk]
            rt_sb = rt_pool.tile([128, cfg.RPAD], BF16, tag="rt", name="rtsb")
            zs_sb = rt_pool.tile([128, cfg.RPAD], BF16, tag="zs", name="zssb")

            def gemm(lhsT_of, w_list, bias, zdst):
                """Z[t] = lhsT_t.T @ W + 1.b ; evac into zs_sb; one DMA out."""
                for t in range(cfg.NB):
                    ps = psum_g.tile([128, 128], F32, tag="gemm_ps", name="ps")
                    for k, (lt, wk) in enumerate(zip(lhsT_of(t), w_list)):
                        nc.tensor.matmul(ps[:], lt, wk,
                                         start=(k == 0), stop=False,
                                         skip_group_check=True)
                    nc.tensor.matmul(ps[:], ones_sb, bias,
                                     start=False, stop=True,
                                     skip_group_check=True)
                    nc.scalar.copy(zs_sb[:, t * 128:(t + 1) * 128], ps[:])
                nc.gpsimd.dma_start(
                    zdst.rearrange("(t p) f -> p t f", p=128)[:, :, :],
                    zs_sb.rearrange("p (t f) -> p t f", f=128)[:, :, :])

            # ---- GEMM1 ----
            def x_lhsT(t):
                tiles = []
                for k in range(cfg.KIN):
                    xt = xt_pool.tile([128, 128], BF16, tag="xt", name="xt")
                    nc.sync.dma_start(
                        xt[:], xt_d[k * 128:(k + 1) * 128,
                                    t * 128:(t + 1) * 128])
                    tiles.append(xt[:])
                return tiles

            gemm(x_lhsT, w1_sb, b1_sb, z1_loc)
            nc.gpsimd.collective_compute(
                "AllGather", mybir.AluOpType.bypass, replica_groups=rg,
                ins=[z1_loc[:, :]], outs=[z1_full[:, :]])

            # ---- SpMM ----
            def spmm(z_full, layer):
                for gi, blist in enumerate(groups):
                    nbl = len(blist)
                    ptiles = [psum_s.tile([128, 128], F32, tag="spmm_ps",
                                          name="spmm_ps")
                              for _ in range(nbl)]

                    def pview(bi):
                        return ptiles[bi][:, :]

                    for q in range(cfg.Q):
                        qi, ioff, n = insts[gi * cfg.Q + q]
                        assert qi == q
                        gb3 = gbuf_pool.tile(
                            [128, cfg.GB * cfg.CAP // 128, 128], BF16,
                            tag="gbuf", name="gbuf")
                        gb = gb3.rearrange("p c f -> p (c f)")
                        # SWDGE ring holds ~1024 descriptors; split gathers
                        o = 0
                        while o < n:
                            nj = min(1024, n - o)
                            nc.gpsimd.dma_gather(
                                out_ap=gb3[:, o // 128:(o + nj) // 128, :],
                                in_ap=z_full[q * cfg.QBASE:
                                             (q + 1) * cfg.QBASE, :],
                                idxs_ap=idx_sb[:, (ioff + o) // 16:
                                               (ioff + o + nj) // 16],
                                num_idxs=nj, num_idxs_reg=nj,
                                elem_size=cfg.HID,
                            )
                            o += nj
                        for bi, b in enumerate(blist):
                            for c in range(cfg.CAP_CH):
                                so = slot_off[(b, q)] - ioff + c * 128
                                cg = (slot_off[(b, q)] + c * 128) // 128
                                s = s_pool.tile([128, 128], BF16, tag="s",
                                                name="s")
                                nc.vector.tensor_scalar(
                                    s[:], iota_sb,
                                    rloc_sb[:, cg:cg + 1],
                                    sval_sb[:, cg:cg + 1],
                                    mybir.AluOpType.is_equal,
                                    mybir.AluOpType.mult)
                                nc.tensor.matmul(
                                    pview(bi), gb[:, so:so + 128], s[:],
                                    start=(q == 0 and c == 0),
                                    stop=(q == cfg.Q - 1 and
                                          c == cfg.CAP_CH - 1),
                                    skip_group_check=True)
                    for pi, pt in enumerate(ptiles):
                        b0 = blist[pi]
                        nw = 128
                        r0 = b0 * 128
                        if layer == 1:
                            nc.scalar.activation(
                                rt_sb[:, r0:r0 + nw], pt[:, :nw],
                                mybir.ActivationFunctionType.Relu)
                        else:
                            ot = ot_pool.tile([128, 512], F32, tag="ot",
                                              name="ot")
                            nc.scalar.copy(ot[:, :nw], pt[:, :nw])
                            nc.sync.dma_start(out_d[:, r0:r0 + nw],
                                              ot[:, :nw])

            spmm(z1_full, 1)

            # ---- GEMM2 ----
            def rt_lhsT(t):
                return [rt_sb[:, t * 128:(t + 1) * 128]]

            gemm(rt_lhsT, [w2_sb], b2_sb, z2_loc)
            nc.gpsimd.collective_compute(
                "AllGather", mybir.AluOpType.bypass, replica_groups=rg,
                ins=[z2_loc[:, :]], outs=[z2_full[:, :]])

            spmm(z2_full, 2)

    nc.compile()
    return nc


def _prep_inputs(cfg, X, W1, b1, W2, b2, per_core, nchunk):
    bf = ml_dtypes.bfloat16
    wcols = cfg.KIN * 128 + 128 + 4 * 128
    wpack = np.zeros((128, wcols), dtype=np.float32)
    for k in range(cfg.KIN):
        wpack[:, k * 128:(k + 1) * 128] = np.asarray(W1)[k * 128:(k + 1) * 128]
    o = cfg.KIN * 128
    wpack[:, o:o + 128] = np.asarray(W2)
    wpack[0, o + 128:o + 256] = np.asarray(b1)
    wpack[0, o + 256:o + 384] = np.asarray(b2)
    wpack[0, o + 384:o + 512] = 1.0
    wpack[:, o + 512:o + 640] = np.arange(128, dtype=np.float32)[None, :]
    wpack = wpack.astype(bf)

    X = np.asarray(X).astype(np.float32)
    in_maps = []
    for m in range(cfg.M):
        xs = np.zeros((cfg.IN, cfg.RPAD), dtype=np.float32)
        xs[:, :cfg.RPC] = X[m * cfg.RPC:(m + 1) * cfg.RPC].T
        fpack = np.zeros((128, 2 * nchunk), dtype=np.float32)
        fpack[:, :nchunk] = per_core[m]["rloc"]
        fpack[:, nchunk:] = per_core[m]["sval"]
        in_maps.append(dict(
            xt=np.ascontiguousarray(xs.astype(bf)), wpack=wpack,
            idx=per_core[m]["idx"], fpack=fpack))
    return in_maps


def run(cfg, X, W1, b1, W2, b2, vals, row, col, trace=False):
    groups, insts, slot_off, nslot, nchunk, per_core, max_seg = \
        build_plan(cfg, row, col, vals)
    nc = build_program(cfg, groups, insts, slot_off, nslot, nchunk)
    in_maps = _prep_inputs(cfg, X, W1, b1, W2, b2, per_core, nchunk)
    res = run_bass_kernel_spmd(nc, in_maps, list(range(cfg.M)), trace=trace)
    outs = [np.asarray(res.results[m]["out"]).T[:cfg.RPC]
            for m in range(cfg.M)]
    out = np.concatenate(outs, axis=0).astype(np.float32)
    return out, res


def kernel(X, W1, b1, W2, b2, vals, row, col):
    out, _ = run(FULL, X, W1, b1, W2, b2, vals, row, col)
    return out



# revision 44
# speedup vs baseline: 62425.0937x; 62425.0937x over previous
"""2-layer GCN on 8 trn2 NeuronCores — commuted, collective-light.

Identity: SpMM(A, X@W + 1b) = (A@X)@W + deg.b  with deg = A@1, so
    H1 = relu((A @ X) @ W1 + deg b1)
    out = (A @ H1) @ W2 + deg b2

Layer 1 (dest-sharded): every core holds the full (replicated) X input,
gathers source rows for its own dest nodes directly - no GEMM before the
aggregation and no collective at all.

Layer 2 (source-sharded): core m owns the edges whose SOURCE col lies in
its range, aggregates partial sums for ALL dest nodes from its LOCAL H1
rows, writes the partial feature-major as [M_dest, 128, RPAD], and a
single bf16 ReduceScatter(add) both sums the partials and delivers each
core its own dest rows.  Collective output is only RPAD*128*2B = 3.2 MB.

SpMM core: per dest-block group, edges sorted by (block, col); slots are
PACKED contiguously across the group's blocks (chunk-aligned only at the
call level).  dma_gather (non-transpose) emits slot-major [128, F]
chunks == matmul lhsT.  S[slot, row] = (iota==rloc)*val, one DVE
tensor_scalar per chunk; straddle chunks span 2+ blocks and get a wider
S plus one matmul per spanned block.  PSUM rule: an accumulation group
(128-col region of a bank) must receive its matmuls consecutively, so
matmuls are ordered region-major.

SPMD: one program for 8 cores; slot layout/capacities are the max over
cores; per-core data varies only in input tensors (pad slots idx=0/val=0).
"""

import sys

import numpy as np
import ml_dtypes

_TRN_REPO = "/opt/trn_rl_repo"
if _TRN_REPO not in sys.path:
    sys.path.insert(0, _TRN_REPO)

import concourse.bass as bass
import concourse.tile as tile
from concourse import bacc, mybir
from concourse.bass_utils import run_bass_kernel_spmd

BF16 = mybir.dt.bfloat16
F32 = mybir.dt.float32
I16 = mybir.dt.int16

GATHER_SPLIT = 4096
NO_RS = False          # max idxs per dma_gather call
SMAX = 3                     # max blocks a chunk may span


class Cfg:
    def __init__(self, n_nodes, in_size, hidden, out_size):
        self.M = 8
        self.NN = n_nodes
        self.IN = in_size
        self.HID = hidden
        self.OUT = out_size
        assert n_nodes % self.M == 0
        self.RPC = n_nodes // self.M          # real rows per core
        self.BL = 128
        self.NB = (self.RPC + 127) // 128     # dest blocks per core (98)
        self.RPAD = self.NB * 128
        self.NNP = self.M * self.RPAD
        self.NBG = self.M * self.NB           # global dest blocks (784)
        self.Q = 4                            # layer-1 col quartiles
        self.QBASE = self.NNP // self.Q
        assert self.QBASE <= 32768
        assert self.RPAD <= 32768             # layer-2 local col space
        self.GB1 = 4                          # layer-1 group: 4 dest blocks
        self.GB2 = 4                          # layer-2 group: 4 dest blocks
        self.KIN = in_size // 128
        assert in_size % 128 == 0 and hidden == 128 and out_size == 128


FULL = Cfg(100000, 256, 128, 128)


def _pack_plan(cfg, groups, nq, ecore):
    """Packed slot plan.

    groups: list of block lists.  nq: #source segments per group (edges of
    (group, q) form one gather call).  ecore[m] = (blk, q, er_loc, idx16,
    val) arrays for core m, where blk is the block's index within the
    GLOBAL block list, er_loc the dest row within the block, idx16 the
    gather index, val the edge weight.

    Returns dict with per-(g,q) call info, per-chunk S specs, per-region
    matmul lists and per-core slot data.
    """
    M = cfg.M
    ngr = len(groups)
    b2g = {}
    b2bi = {}
    for gi, bl in enumerate(groups):
        for bi, b in enumerate(bl):
            b2g[b] = gi
            b2bi[b] = bi

    # per core, per (g, q): ordered edges and per-block prefix boundaries
    seg = [[None] * (ngr * nq) for _ in range(M)]
    for m in range(M):
        blk, q, er, idx, val = ecore[m]
        gi = np.array([b2g[b] for b in blk]) if len(blk) else blk
        key = gi * nq + q
        order = np.lexsort((idx, blk, key))
        ks, bs, es, xs, vs = (key[order], blk[order], er[order],
                              idx[order], val[order])
        kstart = np.searchsorted(ks, np.arange(ngr * nq + 1))
        for k in range(ngr * nq):
            i0, i1 = kstart[k], kstart[k + 1]
            seg[m][k] = (bs[i0:i1], es[i0:i1], xs[i0:i1], vs[i0:i1])

    caps = []
    for k in range(ngr * nq):
        need = max(len(seg[m][k][0]) for m in range(M))
        caps.append(-(-max(need, 1) // 128) * 128)
    ioffs = np.concatenate([[0], np.cumsum(caps)]).astype(np.int64)
    nslot = int(ioffs[-1])
    nchunk = nslot // 128

    idx16 = np.zeros((M, nslot), dtype=np.int16)
    rloc = np.zeros((M, nslot), dtype=np.float32)
    sval = np.zeros((M, nslot), dtype=np.float32)

    chunks = []   # per (g,q): list of (ci, span_first_bi, span_w_blocks)
    regions = []  # per g: {bi: [(q, ci, scol), ...]}
    for gi, bl in enumerate(groups):
        regions.append({bi: [] for bi in range(len(bl))})

    for k in range(ngr * nq):
        gi, q = divmod(k, nq)
        bl = groups[gi]
        cap = caps[k]
        nch = cap // 128
        # per-chunk block span = union over cores
        lo = np.full(nch, 10 ** 9)
        hi = np.full(nch, -1)
        for m in range(M):
            bs, es, xs, vs = seg[m][k]
            n = len(bs)
            if n == 0:
                continue
            base = ioffs[k]
            idx16[m, base:base + n] = xs
            bstart = np.searchsorted(bs, [b for b in bl])
            bend = np.searchsorted(bs, [b + 1 for b in bl])
            for bi in range(len(bl)):
                c0, c1 = bstart[bi] // 128, (bend[bi] - 1) // 128
                if bend[bi] > bstart[bi]:
                    lo[c0:c1 + 1] = np.minimum(lo[c0:c1 + 1], bi)
                    hi[c0:c1 + 1] = np.maximum(hi[c0:c1 + 1], bi)
        lo = np.where(hi < 0, 0, lo)
        hi = np.where(hi < 0, 0, hi)
        # widen: chunk span must be contiguous [lo, hi]
        spans = []
        for c in range(nch):
            w = int(hi[c] - lo[c] + 1)
            assert w <= SMAX, f"span {w} too wide"
            spans.append((int(lo[c]), w))
        chunks.append(spans)
        # rloc relative to span_first
        for m in range(M):
            bs, es, xs, vs = seg[m][k]
            n = len(bs)
            if n == 0:
                continue
            base = ioffs[k]
            sl = np.arange(n)
            spf = np.array([spans[c][0] for c in sl // 128])
            bi_of = np.array([b2bi[b] for b in bs])
            rloc[m, base:base + n] = ((bi_of - spf) * 128 + es)
            sval[m, base:base + n] = vs
        # region matmul lists
        for c in range(nch):
            spf, w = chunks[k][c]
            for bi in range(spf, spf + w):
                if bi < len(bl):
                    regions[gi][bi].append((q, c, (bi - spf) * 128))

    per_core = []
    for m in range(M):
        iw = np.tile(idx16[m].reshape(-1, 16).T, (8, 1))
        rw = rloc[m].reshape(nchunk, 128).T
        sw = sval[m].reshape(nchunk, 128).T
        per_core.append((np.ascontiguousarray(iw),
                         np.ascontiguousarray(rw.astype(np.float32)),
                         np.ascontiguousarray(sw.astype(np.float32))))
    return dict(caps=caps, ioffs=ioffs, nslot=nslot, nchunk=nchunk,
                chunks=chunks, regions=regions, per_core=per_core,
                groups=groups, nq=nq)


def build_plan(cfg, row, col, vals):
    row = np.asarray(row).astype(np.int64)
    col = np.asarray(col).astype(np.int64)
    vals = np.asarray(vals).astype(np.float32)
    cm = col // cfg.RPC
    cr = col % cfg.RPC
    colp = cm * cfg.RPAD + cr               # padded global col id

    # ---- layer 1: dest-sharded, quartile source windows ----
    groups1 = [list(range(g, min(g + cfg.GB1, cfg.NB)))
               for g in range(0, cfg.NB, cfg.GB1)]
    e1 = []
    for m in range(cfg.M):
        sel = (row // cfg.RPC) == m
        er = row[sel] - m * cfg.RPC
        q = colp[sel] // cfg.QBASE
        e1.append((er // 128, q.astype(np.int64),
                   (er % 128).astype(np.int64),
                   (colp[sel] % cfg.QBASE).astype(np.int16),
                   vals[sel]))
    plan1 = _pack_plan(cfg, groups1, cfg.Q, e1)

    # ---- layer 2: source-sharded, global dest blocks ----
    groups2 = []
    for md in range(cfg.M):
        for g in range(0, cfg.NB, cfg.GB2):
            b0 = md * cfg.NB + g
            groups2.append(list(range(b0, b0 + min(cfg.GB2, cfg.NB - g))))
    e2 = []
    for m in range(cfg.M):
        sel = cm == m
        gb = (row[sel] // cfg.RPC) * cfg.NB + (row[sel] % cfg.RPC) // 128
        e2.append((gb, np.zeros(int(sel.sum()), dtype=np.int64),
                   ((row[sel] % cfg.RPC) % 128).astype(np.int64),
                   cr[sel].astype(np.int16),
                   vals[sel]))
    plan2 = _pack_plan(cfg, groups2, 1, e2)

    # deg per dest core
    degs = []
    for m in range(cfg.M):
        sel = (row // cfg.RPC) == m
        er = row[sel] - m * cfg.RPC
        deg = np.zeros(cfg.RPAD, dtype=np.float32)
        np.add.at(deg, er, vals[sel])
        degs.append(deg)
    return plan1, plan2, degs


def build_program(cfg, plan1, plan2):
    nc = bacc.Bacc("TRN2", target_bir_lowering=False, debug=False,
                   num_devices=cfg.M,
                   dynamic_dma_scratch_size=16384,
                   num_swdge_queues=2)

    xfull_d = nc.dram_tensor("xfull", [cfg.NNP, cfg.IN], BF16,
                             kind="ExternalInput")
    wcols = cfg.KIN * 128 + 128 + 2 * 128 + 512
    wpack_d = nc.dram_tensor("wpack", [128, wcols], BF16,
                             kind="ExternalInput")
    idx1_d = nc.dram_tensor("idx1", [128, plan1["nslot"] // 16], I16,
                            kind="ExternalInput")
    idx2_d = nc.dram_tensor("idx2", [128, plan2["nslot"] // 16], I16,
                            kind="ExternalInput")
    fp1_d = nc.dram_tensor("fp1", [128, 2 * plan1["nchunk"]], F32,
                           kind="ExternalInput")
    fp2_d = nc.dram_tensor("fp2", [128, 2 * plan2["nchunk"]], F32,
                           kind="ExternalInput")
    deg_d = nc.dram_tensor("deg", [1, cfg.RPAD], BF16, kind="ExternalInput")
    out_d = nc.dram_tensor("out", [128, cfg.RPAD], F32,
                           kind="ExternalOutput")

    h1_loc = nc.dram_tensor("h1_loc", [cfg.RPAD, cfg.HID], BF16)
    # layer-2 partials, split in two dest-row halves so ReduceScatter #a
    # can fire while the second half is still accumulating
    GA = 15                              # A-half groups per core
    WA = GA * cfg.GB2 * 128              # 7680 cols (blocks 0..59)
    WB = cfg.RPAD - WA                   # 4864 cols (blocks 60..97)
    partA_d = nc.dram_tensor("partA", [cfg.M, 128, WA], BF16)
    partB_d = nc.dram_tensor("partB", [cfg.M, 128, WB], BF16)
    rsA_d = nc.dram_tensor("rsA", [128, WA], BF16)
    rsB_d = nc.dram_tensor("rsB", [128, WB], BF16)

    rg = [list(range(cfg.M))]

    with tile.TileContext(nc) as tc:
        from contextlib import ExitStack
        with ExitStack() as ctx:
            const = ctx.enter_context(tc.tile_pool(name="const", bufs=1))
            gbuf_pool = ctx.enter_context(tc.tile_pool(name="gbuf", bufs=7))
            s1_pool = ctx.enter_context(tc.tile_pool(name="s1", bufs=44))
            s2_pool = ctx.enter_context(tc.tile_pool(name="s2", bufs=10))
            p_pool = ctx.enter_context(tc.tile_pool(name="pb", bufs=6))
            ot_pool = ctx.enter_context(tc.tile_pool(name="ot", bufs=4))
            rt_pool = ctx.enter_context(tc.tile_pool(name="rt", bufs=1))
            psum_s = ctx.enter_context(
                tc.tile_pool(name="psum_s", bufs=6, space="PSUM"))
            psum_h = ctx.enter_context(
                tc.tile_pool(name="psum_h", bufs=2, space="PSUM"))

            # ---- resident constants ----
            wpack_sb = const.tile([128, wcols], BF16, tag="wpack",
                                  name="wpacksb")
            nc.sync.dma_start(wpack_sb[:], wpack_d[:, :])
            w1_sb = [wpack_sb[:, k * 128:(k + 1) * 128]
                     for k in range(cfg.KIN)]
            o = cfg.KIN * 128
            w2_sb = wpack_sb[:, o:o + 128]
            b1_sb = wpack_sb[0:1, o + 128:o + 256]
            b2_sb = wpack_sb[0:1, o + 256:o + 384]
            iota_sb = wpack_sb[:, o + 384:o + 896]
            # idx/fp SBUF is time-shared between the layers: same tag ->
            # same buffer; the layer-2 load waits for layer-1's last reader
            nix = max(plan1["nslot"], plan2["nslot"]) // 16
            nfp = 2 * max(plan1["nchunk"], plan2["nchunk"])
            meta_pool = ctx.enter_context(tc.tile_pool(name="meta", bufs=1))
            idx1_sb = meta_pool.tile([128, nix], I16, tag="idx",
                                     name="idx1sb")
            n16 = plan1["nslot"] // 16
            cut = min(2048, n16)
            nc.sync.dma_start(idx1_sb[:, :cut], idx1_d[:, :cut])
            nc.sync.dma_start(idx1_sb[:, cut:n16], idx1_d[:, cut:])
            fp1_sb = meta_pool.tile([128, nfp], F32, tag="fp",
                                    name="fp1sb")
            nfc = 2 * plan1["nchunk"]
            fcut = min(512, nfc)
            nc.sync.dma_start(fp1_sb[:, :fcut], fp1_d[:, :fcut])
            nc.sync.dma_start(fp1_sb[:, fcut:nfc], fp1_d[:, fcut:])
            deg_sb = const.tile([1, cfg.RPAD], BF16, tag="deg",
                                name="degsb")
            nc.sync.dma_start(deg_sb[:], deg_d[:, :])
            rt_sb = rt_pool.tile([128, cfg.RPAD], BF16, tag="rt",
                                 name="rtsb")

            def spmm_group(plan, gi, idx_sb, rloc_sb, sval_sb, z_of, F,
                           nf, pts):
                """gathers + S builds + region-major chunk matmuls."""
                nq = plan["nq"]
                bl = plan["groups"][gi]
                gts = {}
                for q in range(nq):
                    k = gi * nq + q
                    ioff = int(plan["ioffs"][k])
                    n = plan["caps"][k]
                    gb3 = gbuf_pool.tile([128, n // 128, F], BF16,
                                         tag="gbuf", name="gbuf")
                    gts[q] = gb3
                    o2 = 0
                    while o2 < n:
                        nj = min(GATHER_SPLIT, n - o2)
                        nc.gpsimd.dma_gather(
                            out_ap=gb3[:, o2 // 128:(o2 + nj) // 128, :],
                            in_ap=z_of(q),
                            idxs_ap=idx_sb[:, (ioff + o2) // 16:
                                           (ioff + o2 + nj) // 16],
                            num_idxs=nj, num_idxs_reg=nj,
                            elem_size=F, single_packet=False,
                            queue_num=1)
                        o2 += nj
                # S tiles built lazily at first use (region-major order)
                sdict = {}

                def get_s(q, c):
                    if (q, c) in sdict:
                        return sdict[(q, c)]
                    k = gi * nq + q
                    spf, w = plan["chunks"][k][c]
                    cg = (int(plan["ioffs"][k]) + c * 128) // 128
                    pool = s1_pool if w == 1 else s2_pool
                    s = pool.tile([128, w * 128], BF16,
                                  tag="s1" if w == 1 else "s2", name="s")
                    nc.vector.tensor_scalar(
                        s[:, :], iota_sb[:, :w * 128],
                        rloc_sb[:, cg:cg + 1], sval_sb[:, cg:cg + 1],
                        mybir.AluOpType.is_equal, mybir.AluOpType.mult)
                    sdict[(q, c)] = s
                    return s

                regions = plan["regions"][gi]
                for bi in range(len(bl)):
                    tl = regions[bi]
                    assert tl, f"empty region g{gi} b{bi}"
                    for f in range(nf):
                        for ti, (q, c, scol) in enumerate(tl):
                            s = get_s(q, c)
                            nc.tensor.matmul(
                                pts[f][:, bi * 128:(bi + 1) * 128],
                                gts[q][:, c, f * 128:(f + 1) * 128],
                                s[:, scol:scol + 128],
                                start=(ti == 0), stop=(ti == len(tl) - 1),
                                skip_group_check=True)

            # ================= layer 1 ================================
            nf1 = cfg.IN // 128
            rl1 = fp1_sb[:, 0:plan1["nchunk"]]
            sv1 = fp1_sb[:, plan1["nchunk"]:2 * plan1["nchunk"]]
            for gi, bl in enumerate(plan1["groups"]):
                nbl = len(bl)
                gw = nbl * 128
                r0 = bl[0] * 128
                pts = [psum_s.tile([128, gw], F32, tag="ps1",
                                   name="ps1") for _ in range(nf1)]
                spmm_group(
                    plan1, gi, idx1_sb, rl1, sv1,
                    lambda q: xfull_d[q * cfg.QBASE:(q + 1) * cfg.QBASE, :],
                    cfg.IN, nf1, pts)
                pbs = []
                for f in range(nf1):
                    pb = p_pool.tile([128, gw], BF16, tag=f"p{f}",
                                     name="pb")
                    nc.scalar.copy(pb[:, :], pts[f][:, :])
                    pbs.append(pb)
                hp = psum_h.tile([128, gw], F32, tag="hps", name="hps")
                for bi in range(nbl):
                    hv = hp[:, bi * 128:(bi + 1) * 128]
                    bs = bi * 128
                    for f in range(nf1):
                        nc.tensor.matmul(
                            hv, pbs[f][:, bs:bs + 128], w1_sb[f],
                            start=(f == 0), stop=False,
                            skip_group_check=True)
                    nc.tensor.matmul(
                        hv, deg_sb[:, r0 + bs:r0 + bs + 128], b1_sb,
                        start=False, stop=True, skip_group_check=True)
                nc.scalar.activation(
                    rt_sb[:, r0:r0 + gw], hp[:, :],
                    mybir.ActivationFunctionType.Relu)

            # H1r node-major -> local DRAM (layer-2 gather source).
            # Written in two halves so the first fires under L1's tail.
            h1v = h1_loc.rearrange("(t p) f -> p t f", p=128)
            rtv = rt_sb.rearrange("p (t f) -> p t f", f=128)
            nc.sync.dma_start(h1v[:, 0:49, :], rtv[:, 0:49, :])
            nc.sync.dma_start(h1v[:, 49:, :], rtv[:, 49:, :])

            # ================= layer 2 ================================
            idx2_sb = meta_pool.tile([128, nix], I16, tag="idx",
                                     name="idx2sb")
            nc.sync.dma_start(idx2_sb[:, :plan2["nslot"] // 16],
                              idx2_d[:, :])
            fp2_sb = meta_pool.tile([128, nfp], F32, tag="fp",
                                    name="fp2sb")
            nc.sync.dma_start(fp2_sb[:, :2 * plan2["nchunk"]], fp2_d[:, :])
            rl2 = fp2_sb[:, 0:plan2["nchunk"]]
            sv2 = fp2_sb[:, plan2["nchunk"]:2 * plan2["nchunk"]]
            # emission order: A-half groups (g<12) of every dest core
            # first, then B-half; partial writes staged 4 groups at a time.
            gpc = (cfg.NB + cfg.GB2 - 1) // cfg.GB2   # groups per core (25)
            orderA = [md * gpc + g for md in range(cfg.M)
                      for g in range(15)]
            orderB = [md * gpc + g for md in range(cfg.M)
                      for g in range(15, gpc)]

            def l2_groups(order, half_d, half_w, base_blk):
                stage = None
                s0 = None
                filled = 0
                for gi in order:
                    bl = plan2["groups"][gi]
                    nbl = len(bl)
                    gw = nbl * 128
                    md = bl[0] // cfg.NB
                    bloc = bl[0] - md * cfg.NB - base_blk
                    if stage is None:
                        stage = ot_pool.tile([128, 2048], BF16, tag="st",
                                             name="st")
                        s0 = bloc
                        filled = 0
                    pts = [psum_s.tile([128, gw], F32, tag="ps1",
                                       name="ps1")]
                    spmm_group(plan2, gi, idx2_sb, rl2, sv2,
                               lambda q: h1_loc[:, :], cfg.HID, 1, pts)
                    nc.scalar.copy(stage[:, filled:filled + gw],
                                   pts[0][:, :])
                    filled += gw
                    last_of_core = (gi == order[-1] or
                                    plan2["groups"][order[
                                        order.index(gi) + 1]][0] //
                                    cfg.NB != md)
                    if filled == 2048 or last_of_core:
                        nc.sync.dma_start(
                            half_d[md, :, s0 * 128:s0 * 128 + filled],
                            stage[:, :filled])
                        stage = None

            l2_groups(orderA, partA_d, WA, 0)
            # RS #a is emitted two B-groups in so its (Pool-queue) sem wait
            # does not stall the B gathers behind it
            l2_groups(orderB[:5], partB_d, WB, 60)
            if not NO_RS:
                nc.gpsimd.collective_compute(
                    "ReduceScatter", mybir.AluOpType.add, replica_groups=rg,
                    ins=[partA_d[:, :, :]], outs=[rsA_d[:, :]])
            l2_groups(orderB[5:], partB_d, WB, 60)
            nc.gpsimd.collective_compute(
                "ReduceScatter", mybir.AluOpType.add, replica_groups=rg,
                ins=[partB_d[:, :, :]], outs=[rsB_d[:, :]])

            # ---- post-RS transform: out^T = W2^T P2^T + b2 deg^T ------
            # reuse the (dead) rt buffer for the scattered partial sums
            rsb_full = rt_pool.tile([128, cfg.RPAD], BF16, tag="rt",
                                    name="rsbf")
            for rs_d, w_half, base in ((rsA_d, WA, 0), (rsB_d, WB, 60)):
                rsb = rsb_full[:, base * 128:base * 128 + w_half]
                nc.sync.dma_start(rsb, rs_d[:, :])
                for g in range(0, w_half // 128, 4):
                    nb4 = min(4, w_half // 128 - g)
                    gw = nb4 * 128
                    r0 = (base + g) * 128
                    hp = psum_h.tile([128, gw], F32, tag="hps",
                                     name="hps")
                    nc.tensor.matmul(hp[:, :], w2_sb,
                                     rsb[:, g * 128:g * 128 + gw],
                                     start=True, stop=False,
                                     skip_group_check=True)
                    nc.tensor.matmul(hp[:, :], b2_sb,
                                     deg_sb[:, r0:r0 + gw],
                                     start=False, stop=True,
                                     skip_group_check=True)
                    ot = ot_pool.tile([128, gw], F32, tag="ot", name="ot")
                    nc.scalar.copy(ot[:, :], hp[:, :])
                    nc.sync.dma_start(out_d[:, r0:r0 + gw], ot[:, :])

    nc.compile()
    return nc


def _prep_inputs(cfg, X, W1, b1, W2, b2, plan1, plan2, degs):
    bf = ml_dtypes.bfloat16
    wcols = cfg.KIN * 128 + 128 + 2 * 128 + 512
    wpack = np.zeros((128, wcols), dtype=np.float32)
    for k in range(cfg.KIN):
        wpack[:, k * 128:(k + 1) * 128] = \
            np.asarray(W1)[k * 128:(k + 1) * 128]
    o = cfg.KIN * 128
    wpack[:, o:o + 128] = np.asarray(W2)
    wpack[0, o + 128:o + 256] = np.asarray(b1)
    wpack[0, o + 256:o + 384] = np.asarray(b2)
    wpack[:, o + 384:o + 896] = np.arange(512, dtype=np.float32)[None, :]
    wpack = wpack.astype(bf)

    X = np.asarray(X).astype(np.float32)
    xfull = np.zeros((cfg.NNP, cfg.IN), dtype=bf)
    for m in range(cfg.M):
        xfull[m * cfg.RPAD:m * cfg.RPAD + cfg.RPC] = \
            X[m * cfg.RPC:(m + 1) * cfg.RPC].astype(bf)

    in_maps = []
    for m in range(cfg.M):
        i1, r1, s1 = plan1["per_core"][m]
        i2, r2, s2 = plan2["per_core"][m]
        fp1 = np.concatenate([r1, s1], axis=1)
        fp2 = np.concatenate([r2, s2], axis=1)
        in_maps.append(dict(
            xfull=xfull, wpack=wpack, idx1=i1, idx2=i2,
            fp1=fp1, fp2=fp2,
            deg=degs[m].reshape(1, -1).astype(bf)))
    return in_maps


def run(cfg, X, W1, b1, W2, b2, vals, row, col, trace=False):
    plan1, plan2, degs = build_plan(cfg, row, col, vals)
    nc = build_program(cfg, plan1, plan2)
    in_maps = _prep_inputs(cfg, X, W1, b1, W2, b2, plan1, plan2, degs)
    res = run_bass_kernel_spmd(nc, in_maps, list(range(cfg.M)), trace=trace)
    outs = [np.asarray(res.results[m]["out"]).T[:cfg.RPC]
            for m in range(cfg.M)]
    out = np.concatenate(outs, axis=0).astype(np.float32)
    return out, res


def kernel(X, W1, b1, W2, b2, vals, row, col):
    out, _ = run(FULL, X, W1, b1, W2, b2, vals, row, col)
    return out


# revision 51
# speedup vs baseline: 62878.8495x; 1.0073x over previous
"""2-layer GCN on 8 trn2 NeuronCores — commuted, collective-light.

Identity: SpMM(A, X@W + 1b) = (A@X)@W + deg.b  with deg = A@1, so
    H1 = relu((A @ X) @ W1 + deg b1)
    out = (A @ H1) @ W2 + deg b2

Layer 1 (dest-sharded): every core holds the full (replicated) X input,
gathers source rows for its own dest nodes directly - no GEMM before the
aggregation and no collective at all.

Layer 2 (source-sharded): core m owns the edges whose SOURCE col lies in
its range, aggregates partial sums for ALL dest nodes from its LOCAL H1
rows, writes the partial feature-major as [M_dest, 128, RPAD], and a
single bf16 ReduceScatter(add) both sums the partials and delivers each
core its own dest rows.  Collective output is only RPAD*128*2B = 3.2 MB.

SpMM core: per dest-block group, edges sorted by (block, col); slots are
PACKED contiguously across the group's blocks (chunk-aligned only at the
call level).  dma_gather (non-transpose) emits slot-major [128, F]
chunks == matmul lhsT.  S[slot, row] = (iota==rloc)*val, one DVE
tensor_scalar per chunk; straddle chunks span 2+ blocks and get a wider
S plus one matmul per spanned block.  PSUM rule: an accumulation group
(128-col region of a bank) must receive its matmuls consecutively, so
matmuls are ordered region-major.

SPMD: one program for 8 cores; slot layout/capacities are the max over
cores; per-core data varies only in input tensors (pad slots idx=0/val=0).
"""

import sys

import numpy as np
import ml_dtypes

_TRN_REPO = "/opt/trn_rl_repo"
if _TRN_REPO not in sys.path:
    sys.path.insert(0, _TRN_REPO)

import concourse.bass as bass
import concourse.tile as tile
from concourse import bacc, mybir
from concourse.bass_utils import run_bass_kernel_spmd

BF16 = mybir.dt.bfloat16
F32 = mybir.dt.float32
I16 = mybir.dt.int16

GATHER_SPLIT = 4096    # max idxs per dma_gather call
NO_RS = False          # timing-probe switch: skip collectives (sim only)
SMAX = 3                     # max blocks a chunk may span


class Cfg:
    def __init__(self, n_nodes, in_size, hidden, out_size):
        self.M = 8
        self.NN = n_nodes
        self.IN = in_size
        self.HID = hidden
        self.OUT = out_size
        assert n_nodes % self.M == 0
        self.RPC = n_nodes // self.M          # real rows per core
        self.BL = 128
        self.NB = (self.RPC + 127) // 128     # dest blocks per core (98)
        self.RPAD = self.NB * 128
        self.NNP = self.M * self.RPAD
        self.NBG = self.M * self.NB           # global dest blocks (784)
        self.Q = 4                            # layer-1 col quartiles
        self.QBASE = self.NNP // self.Q
        assert self.QBASE <= 32768
        assert self.RPAD <= 32768             # layer-2 local col space
        self.GB1 = 4                          # layer-1 group: 4 dest blocks
        self.GB2 = 4                          # layer-2 group: 4 dest blocks
        self.KIN = in_size // 128
        assert in_size % 128 == 0 and hidden == 128 and out_size == 128


FULL = Cfg(100000, 256, 128, 128)


def _pack_plan(cfg, groups, nq, ecore):
    """Packed slot plan.

    groups: list of block lists.  nq: #source segments per group (edges of
    (group, q) form one gather call).  ecore[m] = (blk, q, er_loc, idx16,
    val) arrays for core m, where blk is the block's index within the
    GLOBAL block list, er_loc the dest row within the block, idx16 the
    gather index, val the edge weight.

    Returns dict with per-(g,q) call info, per-chunk S specs, per-region
    matmul lists and per-core slot data.
    """
    M = cfg.M
    ngr = len(groups)
    b2g = {}
    b2bi = {}
    for gi, bl in enumerate(groups):
        for bi, b in enumerate(bl):
            b2g[b] = gi
            b2bi[b] = bi

    # per core, per (g, q): ordered edges and per-block prefix boundaries
    seg = [[None] * (ngr * nq) for _ in range(M)]
    for m in range(M):
        blk, q, er, idx, val = ecore[m]
        gi = np.array([b2g[b] for b in blk]) if len(blk) else blk
        key = gi * nq + q
        order = np.lexsort((idx, blk, key))
        ks, bs, es, xs, vs = (key[order], blk[order], er[order],
                              idx[order], val[order])
        kstart = np.searchsorted(ks, np.arange(ngr * nq + 1))
        for k in range(ngr * nq):
            i0, i1 = kstart[k], kstart[k + 1]
            seg[m][k] = (bs[i0:i1], es[i0:i1], xs[i0:i1], vs[i0:i1])

    caps = []
    for k in range(ngr * nq):
        need = max(len(seg[m][k][0]) for m in range(M))
        caps.append(-(-max(need, 1) // 128) * 128)
    ioffs = np.concatenate([[0], np.cumsum(caps)]).astype(np.int64)
    nslot = int(ioffs[-1])
    nchunk = nslot // 128

    idx16 = np.zeros((M, nslot), dtype=np.int16)
    rloc = np.zeros((M, nslot), dtype=np.float32)
    sval = np.zeros((M, nslot), dtype=np.float32)

    chunks = []   # per (g,q): list of (ci, span_first_bi, span_w_blocks)
    regions = []  # per g: {bi: [(q, ci, scol), ...]}
    for gi, bl in enumerate(groups):
        regions.append({bi: [] for bi in range(len(bl))})

    for k in range(ngr * nq):
        gi, q = divmod(k, nq)
        bl = groups[gi]
        cap = caps[k]
        nch = cap // 128
        # per-chunk block span = union over cores
        lo = np.full(nch, 10 ** 9)
        hi = np.full(nch, -1)
        for m in range(M):
            bs, es, xs, vs = seg[m][k]
            n = len(bs)
            if n == 0:
                continue
            base = ioffs[k]
            idx16[m, base:base + n] = xs
            bstart = np.searchsorted(bs, [b for b in bl])
            bend = np.searchsorted(bs, [b + 1 for b in bl])
            for bi in range(len(bl)):
                c0, c1 = bstart[bi] // 128, (bend[bi] - 1) // 128
                if bend[bi] > bstart[bi]:
                    lo[c0:c1 + 1] = np.minimum(lo[c0:c1 + 1], bi)
                    hi[c0:c1 + 1] = np.maximum(hi[c0:c1 + 1], bi)
        lo = np.where(hi < 0, 0, lo)
        hi = np.where(hi < 0, 0, hi)
        # widen: chunk span must be contiguous [lo, hi]
        spans = []
        for c in range(nch):
            w = int(hi[c] - lo[c] + 1)
            assert w <= SMAX, f"span {w} too wide"
            spans.append((int(lo[c]), w))
        chunks.append(spans)
        # rloc relative to span_first
        for m in range(M):
            bs, es, xs, vs = seg[m][k]
            n = len(bs)
            if n == 0:
                continue
            base = ioffs[k]
            sl = np.arange(n)
            spf = np.array([spans[c][0] for c in sl // 128])
            bi_of = np.array([b2bi[b] for b in bs])
            rloc[m, base:base + n] = ((bi_of - spf) * 128 + es)
            sval[m, base:base + n] = vs
        # region matmul lists
        for c in range(nch):
            spf, w = chunks[k][c]
            for bi in range(spf, spf + w):
                if bi < len(bl):
                    regions[gi][bi].append((q, c, (bi - spf) * 128))

    per_core = []
    for m in range(M):
        iw = np.tile(idx16[m].reshape(-1, 16).T, (8, 1))
        rw = rloc[m].reshape(nchunk, 128).T
        sw = sval[m].reshape(nchunk, 128).T
        per_core.append((np.ascontiguousarray(iw),
                         np.ascontiguousarray(rw.astype(np.float32)),
                         np.ascontiguousarray(sw.astype(np.float32))))
    return dict(caps=caps, ioffs=ioffs, nslot=nslot, nchunk=nchunk,
                chunks=chunks, regions=regions, per_core=per_core,
                groups=groups, nq=nq)


def build_plan(cfg, row, col, vals):
    row = np.asarray(row).astype(np.int64)
    col = np.asarray(col).astype(np.int64)
    vals = np.asarray(vals).astype(np.float32)
    cm = col // cfg.RPC
    cr = col % cfg.RPC
    colp = cm * cfg.RPAD + cr               # padded global col id

    # ---- layer 1: dest-sharded, quartile source windows ----
    groups1 = [list(range(g, min(g + cfg.GB1, cfg.NB)))
               for g in range(0, cfg.NB, cfg.GB1)]
    e1 = []
    for m in range(cfg.M):
        sel = (row // cfg.RPC) == m
        er = row[sel] - m * cfg.RPC
        q = colp[sel] // cfg.QBASE
        e1.append((er // 128, q.astype(np.int64),
                   (er % 128).astype(np.int64),
                   (colp[sel] % cfg.QBASE).astype(np.int16),
                   vals[sel]))
    plan1 = _pack_plan(cfg, groups1, cfg.Q, e1)

    # ---- layer 2: source-sharded, global dest blocks ----
    groups2 = []
    for md in range(cfg.M):
        for g in range(0, cfg.NB, cfg.GB2):
            b0 = md * cfg.NB + g
            groups2.append(list(range(b0, b0 + min(cfg.GB2, cfg.NB - g))))
    e2 = []
    for m in range(cfg.M):
        sel = cm == m
        gb = (row[sel] // cfg.RPC) * cfg.NB + (row[sel] % cfg.RPC) // 128
        e2.append((gb, np.zeros(int(sel.sum()), dtype=np.int64),
                   ((row[sel] % cfg.RPC) % 128).astype(np.int64),
                   cr[sel].astype(np.int16),
                   vals[sel]))
    plan2 = _pack_plan(cfg, groups2, 1, e2)

    # deg per dest core
    degs = []
    for m in range(cfg.M):
        sel = (row // cfg.RPC) == m
        er = row[sel] - m * cfg.RPC
        deg = np.zeros(cfg.RPAD, dtype=np.float32)
        np.add.at(deg, er, vals[sel])
        degs.append(deg)
    return plan1, plan2, degs


def build_program(cfg, plan1, plan2):
    nc = bacc.Bacc("TRN2", target_bir_lowering=False, debug=False,
                   num_devices=cfg.M,
                   dynamic_dma_scratch_size=16384,
                   num_swdge_queues=2)

    xfull_d = nc.dram_tensor("xfull", [cfg.NNP, cfg.IN], BF16,
                             kind="ExternalInput")
    wcols = cfg.KIN * 128 + 128 + 2 * 128 + 512
    wpack_d = nc.dram_tensor("wpack", [128, wcols], BF16,
                             kind="ExternalInput")
    idx1_d = nc.dram_tensor("idx1", [128, plan1["nslot"] // 16], I16,
                            kind="ExternalInput")
    idx2_d = nc.dram_tensor("idx2", [128, plan2["nslot"] // 16], I16,
                            kind="ExternalInput")
    fp1_d = nc.dram_tensor("fp1", [128, 2 * plan1["nchunk"]], F32,
                           kind="ExternalInput")
    fp2_d = nc.dram_tensor("fp2", [128, 2 * plan2["nchunk"]], F32,
                           kind="ExternalInput")
    deg_d = nc.dram_tensor("deg", [1, cfg.RPAD], BF16, kind="ExternalInput")
    out_d = nc.dram_tensor("out", [128, cfg.RPAD], F32,
                           kind="ExternalOutput")

    h1_loc = nc.dram_tensor("h1_loc", [cfg.RPAD, cfg.HID], BF16)
    # layer-2 partials, split in two dest-row halves so ReduceScatter #a
    # can fire while the second half is still accumulating
    GA = 18                              # A-half groups per core
    WA = GA * cfg.GB2 * 128              # 9216 cols (blocks 0..71)
    WB = cfg.RPAD - WA                   # 3328 cols (blocks 72..97)
    partA_d = nc.dram_tensor("partA", [cfg.M, 128, WA], BF16)
    partB_d = nc.dram_tensor("partB", [cfg.M, 128, WB], BF16)
    rsA_d = nc.dram_tensor("rsA", [128, WA], BF16)
    rsB_d = nc.dram_tensor("rsB", [128, WB], BF16)

    rg = [list(range(cfg.M))]

    with tile.TileContext(nc) as tc:
        from contextlib import ExitStack
        with ExitStack() as ctx:
            const = ctx.enter_context(tc.tile_pool(name="const", bufs=1))
            gbuf_pool = ctx.enter_context(tc.tile_pool(name="gbuf", bufs=7))
            s1_pool = ctx.enter_context(tc.tile_pool(name="s1", bufs=44))
            s2_pool = ctx.enter_context(tc.tile_pool(name="s2", bufs=10))
            p_pool = ctx.enter_context(tc.tile_pool(name="pb", bufs=6))
            ot_pool = ctx.enter_context(tc.tile_pool(name="ot", bufs=4))
            rt_pool = ctx.enter_context(tc.tile_pool(name="rt", bufs=1))
            psum_s = ctx.enter_context(
                tc.tile_pool(name="psum_s", bufs=6, space="PSUM"))
            psum_h = ctx.enter_context(
                tc.tile_pool(name="psum_h", bufs=2, space="PSUM"))

            # ---- resident constants ----
            wpack_sb = const.tile([128, wcols], BF16, tag="wpack",
                                  name="wpacksb")
            nc.sync.dma_start(wpack_sb[:], wpack_d[:, :])
            w1_sb = [wpack_sb[:, k * 128:(k + 1) * 128]
                     for k in range(cfg.KIN)]
            o = cfg.KIN * 128
            w2_sb = wpack_sb[:, o:o + 128]
            b1_sb = wpack_sb[0:1, o + 128:o + 256]
            b2_sb = wpack_sb[0:1, o + 256:o + 384]
            iota_sb = wpack_sb[:, o + 384:o + 896]
            # idx/fp SBUF is time-shared between the layers: same tag ->
            # same buffer; the layer-2 load waits for layer-1's last reader
            nix = max(plan1["nslot"], plan2["nslot"]) // 16
            nfp = 2 * max(plan1["nchunk"], plan2["nchunk"])
            meta_pool = ctx.enter_context(tc.tile_pool(name="meta", bufs=1))
            idx1_sb = meta_pool.tile([128, nix], I16, tag="idx",
                                     name="idx1sb")
            nc.sync.dma_start(idx1_sb[:, :plan1["nslot"] // 16],
                              idx1_d[:, :])
            fp1_sb = meta_pool.tile([128, nfp], F32, tag="fp",
                                    name="fp1sb")
            nc.sync.dma_start(fp1_sb[:, :2 * plan1["nchunk"]], fp1_d[:, :])
            deg_sb = const.tile([1, cfg.RPAD], BF16, tag="deg",
                                name="degsb")
            nc.sync.dma_start(deg_sb[:], deg_d[:, :])
            rt_sb = rt_pool.tile([128, cfg.RPAD], BF16, tag="rt",
                                 name="rtsb")

            def spmm_group(plan, gi, idx_sb, rloc_sb, sval_sb, z_of, F,
                           nf, pts):
                """gathers + S builds + region-major chunk matmuls."""
                nq = plan["nq"]
                bl = plan["groups"][gi]
                gts = {}
                for q in range(nq):
                    k = gi * nq + q
                    ioff = int(plan["ioffs"][k])
                    n = plan["caps"][k]
                    gb3 = gbuf_pool.tile([128, n // 128, F], BF16,
                                         tag="gbuf", name="gbuf")
                    gts[q] = gb3
                    o2 = 0
                    while o2 < n:
                        nj = min(GATHER_SPLIT, n - o2)
                        nc.gpsimd.dma_gather(
                            out_ap=gb3[:, o2 // 128:(o2 + nj) // 128, :],
                            in_ap=z_of(q),
                            idxs_ap=idx_sb[:, (ioff + o2) // 16:
                                           (ioff + o2 + nj) // 16],
                            num_idxs=nj, num_idxs_reg=nj,
                            elem_size=F, single_packet=False,
                            queue_num=1)
                        o2 += nj
                # S tiles built lazily at first use (region-major order)
                sdict = {}

                def get_s(q, c):
                    if (q, c) in sdict:
                        return sdict[(q, c)]
                    k = gi * nq + q
                    spf, w = plan["chunks"][k][c]
                    cg = (int(plan["ioffs"][k]) + c * 128) // 128
                    pool = s1_pool if w == 1 else s2_pool
                    s = pool.tile([128, w * 128], BF16,
                                  tag="s1" if w == 1 else "s2", name="s")
                    nc.vector.tensor_scalar(
                        s[:, :], iota_sb[:, :w * 128],
                        rloc_sb[:, cg:cg + 1], sval_sb[:, cg:cg + 1],
                        mybir.AluOpType.is_equal, mybir.AluOpType.mult)
                    sdict[(q, c)] = s
                    return s

                regions = plan["regions"][gi]
                for bi in range(len(bl)):
                    tl = regions[bi]
                    assert tl, f"empty region g{gi} b{bi}"
                    for f in range(nf):
                        for ti, (q, c, scol) in enumerate(tl):
                            s = get_s(q, c)
                            nc.tensor.matmul(
                                pts[f][:, bi * 128:(bi + 1) * 128],
                                gts[q][:, c, f * 128:(f + 1) * 128],
                                s[:, scol:scol + 128],
                                start=(ti == 0), stop=(ti == len(tl) - 1),
                                skip_group_check=True)

            # ================= layer 1 ================================
            nf1 = cfg.IN // 128
            rl1 = fp1_sb[:, 0:plan1["nchunk"]]
            sv1 = fp1_sb[:, plan1["nchunk"]:2 * plan1["nchunk"]]
            for gi, bl in enumerate(plan1["groups"]):
                nbl = len(bl)
                gw = nbl * 128
                r0 = bl[0] * 128
                pts = [psum_s.tile([128, gw], F32, tag="ps1",
                                   name="ps1") for _ in range(nf1)]
                spmm_group(
                    plan1, gi, idx1_sb, rl1, sv1,
                    lambda q: xfull_d[q * cfg.QBASE:(q + 1) * cfg.QBASE, :],
                    cfg.IN, nf1, pts)
                pbs = []
                for f in range(nf1):
                    pb = p_pool.tile([128, gw], BF16, tag=f"p{f}",
                                     name="pb")
                    nc.scalar.copy(pb[:, :], pts[f][:, :])
                    pbs.append(pb)
                hp = psum_h.tile([128, gw], F32, tag="hps", name="hps")
                for bi in range(nbl):
                    hv = hp[:, bi * 128:(bi + 1) * 128]
                    bs = bi * 128
                    for f in range(nf1):
                        nc.tensor.matmul(
                            hv, pbs[f][:, bs:bs + 128], w1_sb[f],
                            start=(f == 0), stop=False,
                            skip_group_check=True)
                    nc.tensor.matmul(
                        hv, deg_sb[:, r0 + bs:r0 + bs + 128], b1_sb,
                        start=False, stop=True, skip_group_check=True)
                nc.scalar.activation(
                    rt_sb[:, r0:r0 + gw], hp[:, :],
                    mybir.ActivationFunctionType.Relu)

            # H1r node-major -> local DRAM (layer-2 gather source).
            # Written in two halves so the first fires under L1's tail.
            h1v = h1_loc.rearrange("(t p) f -> p t f", p=128)
            rtv = rt_sb.rearrange("p (t f) -> p t f", f=128)
            nc.sync.dma_start(h1v[:, 0:49, :], rtv[:, 0:49, :])
            nc.sync.dma_start(h1v[:, 49:, :], rtv[:, 49:, :])

            # ================= layer 2 ================================
            idx2_sb = meta_pool.tile([128, nix], I16, tag="idx",
                                     name="idx2sb")
            nc.sync.dma_start(idx2_sb[:, :plan2["nslot"] // 16],
                              idx2_d[:, :])
            fp2_sb = meta_pool.tile([128, nfp], F32, tag="fp",
                                    name="fp2sb")
            nc.sync.dma_start(fp2_sb[:, :2 * plan2["nchunk"]], fp2_d[:, :])
            rl2 = fp2_sb[:, 0:plan2["nchunk"]]
            sv2 = fp2_sb[:, plan2["nchunk"]:2 * plan2["nchunk"]]
            # emission order: A-half groups (g<12) of every dest core
            # first, then B-half; partial writes staged 4 groups at a time.
            gpc = (cfg.NB + cfg.GB2 - 1) // cfg.GB2   # groups per core (25)
            orderA = [md * gpc + g for md in range(cfg.M)
                      for g in range(18)]
            orderB = [md * gpc + g for md in range(cfg.M)
                      for g in range(18, gpc)]

            def l2_groups(order, half_d, half_w, base_blk):
                stage = None
                s0 = None
                filled = 0
                for gi in order:
                    bl = plan2["groups"][gi]
                    nbl = len(bl)
                    gw = nbl * 128
                    md = bl[0] // cfg.NB
                    bloc = bl[0] - md * cfg.NB - base_blk
                    if stage is None:
                        stage = ot_pool.tile([128, 2048], BF16, tag="st",
                                             name="st")
                        s0 = bloc
                        filled = 0
                    pts = [psum_s.tile([128, gw], F32, tag="ps1",
                                       name="ps1")]
                    spmm_group(plan2, gi, idx2_sb, rl2, sv2,
                               lambda q: h1_loc[:, :], cfg.HID, 1, pts)
                    nc.scalar.copy(stage[:, filled:filled + gw],
                                   pts[0][:, :])
                    filled += gw
                    last_of_core = (gi == order[-1] or
                                    plan2["groups"][order[
                                        order.index(gi) + 1]][0] //
                                    cfg.NB != md)
                    if filled == 2048 or last_of_core:
                        nc.sync.dma_start(
                            half_d[md, :, s0 * 128:s0 * 128 + filled],
                            stage[:, :filled])
                        stage = None

            l2_groups(orderA, partA_d, WA, 0)
            # RS #a is emitted two B-groups in so its (Pool-queue) sem wait
            # does not stall the B gathers behind it
            l2_groups(orderB[:5], partB_d, WB, 72)
            if not NO_RS:
                nc.gpsimd.collective_compute(
                    "ReduceScatter", mybir.AluOpType.add, replica_groups=rg,
                    ins=[partA_d[:, :, :]], outs=[rsA_d[:, :]])
            l2_groups(orderB[5:], partB_d, WB, 72)
            nc.gpsimd.collective_compute(
                "ReduceScatter", mybir.AluOpType.add, replica_groups=rg,
                ins=[partB_d[:, :, :]], outs=[rsB_d[:, :]])

            # ---- post-RS transform: out^T = W2^T P2^T + b2 deg^T ------
            # reuse the (dead) rt buffer for the scattered partial sums
            rsb_full = rt_pool.tile([128, cfg.RPAD], BF16, tag="rt",
                                    name="rsbf")
            for rs_d, w_half, base in ((rsA_d, WA, 0), (rsB_d, WB, 72)):
                rsb = rsb_full[:, base * 128:base * 128 + w_half]
                nc.sync.dma_start(rsb, rs_d[:, :])
                for g in range(0, w_half // 128, 4):
                    nb4 = min(4, w_half // 128 - g)
                    gw = nb4 * 128
                    r0 = (base + g) * 128
                    hp = psum_h.tile([128, gw], F32, tag="hps",
                                     name="hps")
                    nc.tensor.matmul(hp[:, :], w2_sb,
                                     rsb[:, g * 128:g * 128 + gw],
                                     start=True, stop=False,
                                     skip_group_check=True)
                    nc.tensor.matmul(hp[:, :], b2_sb,
                                     deg_sb[:, r0:r0 + gw],
                                     start=False, stop=True,
                                     skip_group_check=True)
                    ot = ot_pool.tile([128, gw], F32, tag="ot", name="ot")
                    nc.scalar.copy(ot[:, :], hp[:, :])
                    nc.sync.dma_start(out_d[:, r0:r0 + gw], ot[:, :])

    nc.compile()
    return nc


def _prep_inputs(cfg, X, W1, b1, W2, b2, plan1, plan2, degs):
    bf = ml_dtypes.bfloat16
    wcols = cfg.KIN * 128 + 128 + 2 * 128 + 512
    wpack = np.zeros((128, wcols), dtype=np.float32)
    for k in range(cfg.KIN):
        wpack[:, k * 128:(k + 1) * 128] = \
            np.asarray(W1)[k * 128:(k + 1) * 128]
    o = cfg.KIN * 128
    wpack[:, o:o + 128] = np.asarray(W2)
    wpack[0, o + 128:o + 256] = np.asarray(b1)
    wpack[0, o + 256:o + 384] = np.asarray(b2)
    wpack[:, o + 384:o + 896] = np.arange(512, dtype=np.float32)[None, :]
    wpack = wpack.astype(bf)

    X = np.asarray(X).astype(np.float32)
    xfull = np.zeros((cfg.NNP, cfg.IN), dtype=bf)
    for m in range(cfg.M):
        xfull[m * cfg.RPAD:m * cfg.RPAD + cfg.RPC] = \
            X[m * cfg.RPC:(m + 1) * cfg.RPC].astype(bf)

    in_maps = []
    for m in range(cfg.M):
        i1, r1, s1 = plan1["per_core"][m]
        i2, r2, s2 = plan2["per_core"][m]
        fp1 = np.concatenate([r1, s1], axis=1)
        fp2 = np.concatenate([r2, s2], axis=1)
        in_maps.append(dict(
            xfull=xfull, wpack=wpack, idx1=i1, idx2=i2,
            fp1=fp1, fp2=fp2,
            deg=degs[m].reshape(1, -1).astype(bf)))
    return in_maps


def run(cfg, X, W1, b1, W2, b2, vals, row, col, trace=False):
    plan1, plan2, degs = build_plan(cfg, row, col, vals)
    nc = build_program(cfg, plan1, plan2)
    in_maps = _prep_inputs(cfg, X, W1, b1, W2, b2, plan1, plan2, degs)
    res = run_bass_kernel_spmd(nc, in_maps, list(range(cfg.M)), trace=trace)
    outs = [np.asarray(res.results[m]["out"]).T[:cfg.RPC]
            for m in range(cfg.M)]
    out = np.concatenate(outs, axis=0).astype(np.float32)
    return out, res


def kernel(X, W1, b1, W2, b2, vals, row, col):
    out, _ = run(FULL, X, W1, b1, W2, b2, vals, row, col)
    return out
